# revision 12
# baseline (speedup 1.0000x reference)
"""NT-Xent contrastive loss on 8 Trainium2 NeuronCores.

Reference computation (B=4096, D=128, T=0.5):
    z = row-normalize(concat(emb_i, emb_j))           # [8192, 128]
    sim = z @ z.T                                     # [8192, 8192]
    S_r = sum_l exp(sim[r,l]/T),  denom_r = S_r - exp(sim[r,r]/T)
    pos_r = sim[r, r+-B]
    loss = mean_r ( log(denom_r) - pos_r/T )

Sharding: rows of sim are split 1024-per-core (8 cores).  Every core gets
the full raw reps (the "all-gather"), normalizes + transposes them into a
bf16 Z^T [128d, 8192rows] in SBUF, and computes its 1024-row strip of
exp(sim/T) row-sums with PE matmuls + ScalarE Exp(accum).  Positives are
computed in fp32 from per-core natural-layout row blocks (rows_a = own
rows, rows_b = partner rows), so the SPMD program itself is
core-independent.  Each core emits [128, 8] per-row loss terms; the host
sums them and divides by 2B.

Numerics: the big Gram matrix runs in bf16 (PE) with fp32 PSUM accum; the
diagonal term is subtracted as the constant e^2 (sim[r,r] = 1 +- 2e-3 in
bf16 -> error ~4e-6 relative on the denominator).  Norms use
exp(-0.5*ln(s)) instead of sqrt so every ScalarE op lives in the single
"natural_log_exp_and_others" activation-table set (no 2.7us table swaps).
"""

import math

import numpy as np

import concourse.bass as bass
import concourse.mybir as mybir
import concourse.tile as tile
from concourse import masks
from concourse.bass_utils import run_bass_kernel_spmd

B = 4096
D = 128
NR = 2 * B               # 8192 rows of reps / sim
N_CORES = 8
RPC = NR // N_CORES      # 1024 rows per core
P = 128                  # partitions
NG = 8                   # row groups of 1024 (also zT column groups)
MT = RPC // P            # 8 local row tiles per core
TEMPERATURE = 0.5
INV_T = 1.0 / TEMPERATURE          # 2.0
E2 = math.exp(1.0 / TEMPERATURE)   # exp(sim_rr / T), sim_rr == 1

_NC = None
TRACE = False            # test.py flips this for profiled runs
_LAST_RESULT = None      # test.py reads exec_time_ns / trace from here

f32 = mybir.dt.float32
bf16 = mybir.dt.bfloat16
AF = mybir.ActivationFunctionType
OP = mybir.AluOpType


def _patched_clear_and_free_semaphores(self, sems):
    """Replacement for Bass.clear_and_free_semaphores: the stock version
    emits a raw-ISA EVENT_SEMAPHORE_RANGE_CLEAR that this toolchain's walrus
    rejects ("ISA wrong length").  Emit BIR-native per-sem `wr-imm 0`
    updates on gpsimd NOPs instead — same semantics (sems reset between
    NEFF executions), supported lowering."""
    if not sems:
        return
    sem_nums = [s.num if hasattr(s, "num") else s for s in sems]
    for n in sem_nums:
        inst = self.gpsimd.nop()
        upd = mybir.SyncUpdate(
            sync_type="semaphore",
            id=n,
            update_mode="sem-wr-imm",
            update_value=0,
            ant_name=f"semclr{n}",
        )
        si = inst.ins.sync_info
        if si is None:
            inst.ins.sync_info = mybir.SyncInfo(on_wait=[], on_update=[upd])
        else:
            si.on_update.append(upd)
    self._state.prepend_free_semaphores(sem_nums)
    for poison_set in self._tile_sem_poison_stack:
        poison_set.update(sem_nums)


def _hoist_excess_waits(nc):
    """This toolchain's walrus (CoreV3GenImpl) allows only ONE sync-wait on
    most compute instruction structs; Tile sometimes attaches two.  Hoist
    all-but-one wait onto same-engine EventSemaphore carriers (2 wait slots
    each) inserted immediately before the instruction — same-engine program
    order makes this semantically identical."""
    n = 0
    for f in nc.m.functions:
        for blk in f.blocks:
            out = []
            for inst in blk.instructions:
                si = inst.sync_info
                tn = type(inst).__name__
                if (
                    si is not None
                    and len(si.on_wait) > 1
                    and tn != "InstEventSemaphore"
                ):
                    waits = list(si.on_wait)
                    keep, extra = waits[-1:], waits[:-1]
                    while extra:
                        grp, extra = extra[:2], extra[2:]
                        es = mybir.InstEventSemaphore(
                            name=f"wcarrier_{n}", ins=[], outs=[]
                        )
                        n += 1
                        es.engine = inst.engine
                        es.sync_info = mybir.SyncInfo(on_wait=list(grp), on_update=[])
                        out.append(es)
                    inst.sync_info = mybir.SyncInfo(
                        on_wait=keep, on_update=list(si.on_update)
                    )
                out.append(inst)
            blk.instructions[:] = out


def _build_nc() -> bass.Bass:
    nc = bass.Bass("TRN2", target_bir_lowering=False, debug=False)
    import types as _types

    nc.clear_and_free_semaphores = _types.MethodType(
        _patched_clear_and_free_semaphores, nc
    )

    reps = nc.dram_tensor("reps", [NR, D], f32, kind="ExternalInput")
    rows_a = nc.dram_tensor("rows_a", [RPC, D], f32, kind="ExternalInput")
    rows_b = nc.dram_tensor("rows_b", [RPC, D], f32, kind="ExternalInput")
    out_d = nc.dram_tensor("out", [P, MT], f32, kind="ExternalOutput")

    with tile.TileContext(nc) as tc:
        with (
            tc.tile_pool(name="singles", bufs=1) as singles,
            tc.tile_pool(name="loads", bufs=3) as loads,
            tc.tile_pool(name="small", bufs=4) as small,
            tc.tile_pool(name="scratch", bufs=2) as scratch,
            tc.tile_pool(name="psum_t", bufs=2, space="PSUM") as psum_t,
            tc.tile_pool(name="psum_mm", bufs=3, space="PSUM") as psum_mm,
        ):
            ident = singles.tile([P, P], f32, tag="ident")
            masks.make_identity(nc, ident[:])

            # persistent SBUF buffers
            zT = [
                singles.tile([P, RPC], bf16, name=f"zT{g}", tag=f"zT{g}")
                for g in range(NG)
            ]
            lhsT = singles.tile([P, RPC], bf16, tag="lhsT")
            zA = singles.tile([P, RPC], f32, tag="zA")
            zB = singles.tile([P, RPC], f32, tag="zB")
            ss_ab = singles.tile([P, 2 * MT], f32, tag="ss_ab")
            lns_ab = singles.tile([P, 2 * MT], f32, tag="lns_ab")
            inv_ab = singles.tile([P, 2 * MT], f32, tag="inv_ab")
            esums = singles.tile([P, MT * NG], f32, tag="esums")
            pos = singles.tile([P, MT], f32, tag="pos")
            svec = singles.tile([P, MT], f32, tag="svec")
            denoms = singles.tile([P, MT], f32, tag="denoms")
            lnb = singles.tile([P, MT], f32, tag="lnb")
            pos2 = singles.tile([P, MT], f32, tag="pos2")
            outb = singles.tile([P, MT], f32, tag="outb")

            # ---- load own + partner row blocks (natural layout) ----
            # row r = t*128 + p  ->  zA[p, t*128:(t+1)*128]
            nc.sync.dma_start(
                out=zA[:].rearrange("p (n d) -> p n d", d=D),
                in_=rows_a.ap().rearrange("(n p) d -> p n d", p=P),
            )
            nc.sync.dma_start(
                out=zB[:].rearrange("p (n d) -> p n d", d=D),
                in_=rows_b.ap().rearrange("(n p) d -> p n d", p=P),
            )

            # ---- normalize A/B in fp32 ----
            for t in range(2 * MT):
                src = zA if t < MT else zB
                sl = slice((t % MT) * D, (t % MT + 1) * D)
                scr = scratch.tile([P, D], f32, tag="scr")
                nc.vector.tensor_mul(scr[:], src[:, sl], src[:, sl])
                nc.vector.tensor_reduce(
                    ss_ab[:, t : t + 1], scr[:], axis=mybir.AxisListType.X, op=OP.add
                )
            nc.scalar.activation(lns_ab[:], ss_ab[:], AF.Ln)
            nc.scalar.activation(inv_ab[:], lns_ab[:], AF.Exp, scale=-0.5)
            for t in range(2 * MT):
                src = zA if t < MT else zB
                sl = slice((t % MT) * D, (t % MT + 1) * D)
                nc.vector.tensor_scalar_mul(src[:, sl], src[:, sl], inv_ab[:, t : t + 1])

            for m in range(MT):
                sl = slice(m * D, (m + 1) * D)
                # positives: pos[p, m] = sum_d zA[p, m, d] * zB[p, m, d]
                scr = scratch.tile([P, D], f32, tag="scr")
                nc.vector.tensor_mul(scr[:], zA[:, sl], zB[:, sl])
                nc.vector.tensor_reduce(
                    pos[:, m : m + 1], scr[:], axis=mybir.AxisListType.X, op=OP.add
                )
                # lhsT[:, m*128+j] = zA row j of tile m (transposed, cast bf16)
                pt = psum_t.tile([P, P], f32, tag="pt")
                nc.tensor.transpose(pt[:], zA[:, sl], ident[:])
                nc.vector.tensor_copy(lhsT[:, sl], pt[:])

            # ---- main pipeline over 8 groups of 1024 reps rows ----
            reps_v = reps.ap().rearrange("(g n p) d -> g p n d", g=NG, p=P)
            for g in range(NG):
                ld = loads.tile([P, RPC], f32, tag="ld")
                nc.sync.dma_start(
                    out=ld[:].rearrange("p (n d) -> p n d", d=D), in_=reps_v[g]
                )
                ss = small.tile([P, MT], f32, tag="ss")
                for t in range(MT):
                    sl = slice(t * D, (t + 1) * D)
                    scr = scratch.tile([P, D], f32, tag="scr")
                    nc.vector.tensor_mul(scr[:], ld[:, sl], ld[:, sl])
                    nc.vector.tensor_reduce(
                        ss[:, t : t + 1], scr[:], axis=mybir.AxisListType.X, op=OP.add
                    )
                lns = small.tile([P, MT], f32, tag="lns")
                nc.scalar.activation(lns[:], ss[:], AF.Ln)
                inv = small.tile([P, MT], f32, tag="inv")
                nc.scalar.activation(inv[:], lns[:], AF.Exp, scale=-0.5)
                for t in range(MT):
                    sl = slice(t * D, (t + 1) * D)
                    nc.vector.tensor_scalar_mul(ld[:, sl], ld[:, sl], inv[:, t : t + 1])
                    pt = psum_t.tile([P, P], f32, tag="pt")
                    nc.tensor.transpose(pt[:], ld[:, sl], ident[:])
                    nc.vector.tensor_copy(zT[g][:, sl], pt[:])

                # local sim strip against this column group
                for m in range(MT):
                    msl = slice(m * D, (m + 1) * D)
                    pg = psum_mm.tile([P, 1024], f32, tag="pg")
                    nc.tensor.matmul(
                        pg[:, 0:512], lhsT[:, msl], zT[g][:, 0:512],
                        start=True, stop=True,
                    )
                    nc.tensor.matmul(
                        pg[:, 512:1024], lhsT[:, msl], zT[g][:, 512:1024],
                        start=True, stop=True,
                    )
                    # exp(sim/T) in place on PSUM; row-sum into esums column
                    nc.scalar.activation(
                        pg[:], pg[:], AF.Exp, scale=INV_T,
                        accum_out=esums[:, m * NG + g : m * NG + g + 1],
                    )

            # ---- finale: loss terms per local row ----
            for m in range(MT):
                nc.vector.tensor_reduce(
                    svec[:, m : m + 1], esums[:, m * NG : (m + 1) * NG],
                    axis=mybir.AxisListType.X, op=OP.add,
                )
            nc.vector.tensor_scalar_add(denoms[:], svec[:], -E2)
            nc.scalar.activation(lnb[:], denoms[:], AF.Ln)
            nc.vector.tensor_scalar_mul(pos2[:], pos[:], INV_T)
            nc.vector.tensor_tensor(outb[:], lnb[:], pos2[:], OP.subtract)
            nc.sync.dma_start(out=out_d.ap(), in_=outb[:])

    _hoist_excess_waits(nc)
    return nc


def _get_nc() -> bass.Bass:
    global _NC
    if _NC is None:
        _NC = _build_nc()
    return _NC


def kernel(emb_i: np.ndarray, emb_j: np.ndarray) -> np.ndarray:
    global _LAST_RESULT
    reps = np.ascontiguousarray(
        np.concatenate(
            [np.asarray(emb_i, np.float32), np.asarray(emb_j, np.float32)], axis=0
        )
    )
    assert reps.shape == (NR, D)

    in_maps = []
    for c in range(N_CORES):
        lo = c * RPC
        pa = (lo + B) % NR
        in_maps.append(
            {
                "reps": reps,
                "rows_a": np.ascontiguousarray(reps[lo : lo + RPC]),
                "rows_b": np.ascontiguousarray(reps[pa : pa + RPC]),
            }
        )

    kw = {}
    if TRACE:
        import os
        import tempfile

        kw["tmpdir"] = tempfile.mkdtemp(prefix="trace_", dir=os.getcwd())
    res = run_bass_kernel_spmd(
        _get_nc(), in_maps, list(range(N_CORES)), trace=TRACE, **kw
    )
    _LAST_RESULT = res

    total = 0.0
    for r in res.results:
        total += float(np.asarray(r["out"], dtype=np.float64).sum())
    return np.asarray(np.float32(total / NR))


# revision 13
# speedup vs baseline: 1.0456x; 1.0456x over previous
"""NT-Xent contrastive loss on 8 Trainium2 NeuronCores.

Reference computation (B=4096, D=128, T=0.5):
    z = row-normalize(concat(emb_i, emb_j))           # [8192, 128]
    sim = z @ z.T                                     # [8192, 8192]
    S_r = sum_l exp(sim[r,l]/T),  denom_r = S_r - exp(sim[r,r]/T)
    pos_r = sim[r, r+-B]
    loss = mean_r ( log(denom_r) - pos_r/T )

Sharding: rows of sim are split 1024-per-core (8 cores).  Every core gets
the full raw reps (the "all-gather"), normalizes + transposes them into a
bf16 Z^T [128d, 8192rows] in SBUF, and computes its 1024-row strip of
exp(sim/T) row-sums with PE matmuls + ScalarE Exp(accum).  Positives are
computed in fp32 from per-core natural-layout row blocks (rows_a = own
rows, rows_b = partner rows), so the SPMD program itself is
core-independent.  Each core emits [128, 8] per-row loss terms; the host
sums them and divides by 2B.

Numerics: the big Gram matrix runs in bf16 (PE) with fp32 PSUM accum; the
diagonal term is subtracted as the constant e^2 (sim[r,r] = 1 +- 2e-3 in
bf16 -> error ~4e-6 relative on the denominator).  Norms use
exp(-0.5*ln(s)) instead of sqrt so every ScalarE op lives in the single
"natural_log_exp_and_others" activation-table set (no 2.7us table swaps).
"""

import math

import numpy as np

import concourse.bass as bass
import concourse.mybir as mybir
import concourse.tile as tile
from concourse import masks
from concourse.bass_utils import run_bass_kernel_spmd

B = 4096
D = 128
NR = 2 * B               # 8192 rows of reps / sim
N_CORES = 8
RPC = NR // N_CORES      # 1024 rows per core
P = 128                  # partitions
NG = 8                   # row groups of 1024 (also zT column groups)
MT = RPC // P            # 8 local row tiles per core
TEMPERATURE = 0.5
INV_T = 1.0 / TEMPERATURE          # 2.0
E2 = math.exp(1.0 / TEMPERATURE)   # exp(sim_rr / T), sim_rr == 1

_NC = None
TRACE = False            # test.py flips this for profiled runs
_LAST_RESULT = None      # test.py reads exec_time_ns / trace from here

f32 = mybir.dt.float32
bf16 = mybir.dt.bfloat16
AF = mybir.ActivationFunctionType
OP = mybir.AluOpType


def _patched_clear_and_free_semaphores(self, sems):
    """Replacement for Bass.clear_and_free_semaphores: the stock version
    emits a raw-ISA EVENT_SEMAPHORE_RANGE_CLEAR that this toolchain's walrus
    rejects ("ISA wrong length").  Emit BIR-native per-sem `wr-imm 0`
    updates on gpsimd NOPs instead — same semantics (sems reset between
    NEFF executions), supported lowering."""
    if not sems:
        return
    sem_nums = [s.num if hasattr(s, "num") else s for s in sems]
    for n in sem_nums:
        inst = self.gpsimd.nop()
        upd = mybir.SyncUpdate(
            sync_type="semaphore",
            id=n,
            update_mode="sem-wr-imm",
            update_value=0,
            ant_name=f"semclr{n}",
        )
        si = inst.ins.sync_info
        if si is None:
            inst.ins.sync_info = mybir.SyncInfo(on_wait=[], on_update=[upd])
        else:
            si.on_update.append(upd)
    self._state.prepend_free_semaphores(sem_nums)
    for poison_set in self._tile_sem_poison_stack:
        poison_set.update(sem_nums)


def _hoist_excess_waits(nc):
    """This toolchain's walrus (CoreV3GenImpl) allows only ONE sync-wait on
    most compute instruction structs; Tile sometimes attaches two.  Hoist
    all-but-one wait onto same-engine EventSemaphore carriers (2 wait slots
    each) inserted immediately before the instruction — same-engine program
    order makes this semantically identical."""
    n = 0
    for f in nc.m.functions:
        for blk in f.blocks:
            out = []
            for inst in blk.instructions:
                si = inst.sync_info
                tn = type(inst).__name__
                if (
                    si is not None
                    and len(si.on_wait) > 1
                    and tn != "InstEventSemaphore"
                ):
                    waits = list(si.on_wait)
                    keep, extra = waits[-1:], waits[:-1]
                    while extra:
                        grp, extra = extra[:2], extra[2:]
                        es = mybir.InstEventSemaphore(
                            name=f"wcarrier_{n}", ins=[], outs=[]
                        )
                        n += 1
                        es.engine = inst.engine
                        es.sync_info = mybir.SyncInfo(on_wait=list(grp), on_update=[])
                        out.append(es)
                    inst.sync_info = mybir.SyncInfo(
                        on_wait=keep, on_update=list(si.on_update)
                    )
                out.append(inst)
            blk.instructions[:] = out


def _build_nc() -> bass.Bass:
    nc = bass.Bass("TRN2", target_bir_lowering=False, debug=False)
    import types as _types

    nc.clear_and_free_semaphores = _types.MethodType(
        _patched_clear_and_free_semaphores, nc
    )

    reps = nc.dram_tensor("reps", [NR, D], f32, kind="ExternalInput")
    rows_a = nc.dram_tensor("rows_a", [RPC, D], f32, kind="ExternalInput")
    rows_b = nc.dram_tensor("rows_b", [RPC, D], f32, kind="ExternalInput")
    out_d = nc.dram_tensor("out", [P, MT], f32, kind="ExternalOutput")

    with tile.TileContext(nc) as tc:
        with (
            tc.tile_pool(name="singles", bufs=1) as singles,
            tc.tile_pool(name="loads", bufs=3) as loads,
            tc.tile_pool(name="small", bufs=4) as small,
            tc.tile_pool(name="scratch", bufs=2) as scratch,
            tc.tile_pool(name="psum_t", bufs=2, space="PSUM") as psum_t,
            tc.tile_pool(name="psum_mm", bufs=3, space="PSUM") as psum_mm,
        ):
            ident = singles.tile([P, P], f32, tag="ident")
            masks.make_identity(nc, ident[:])

            # persistent SBUF buffers
            zT = [
                singles.tile([P, RPC], bf16, name=f"zT{g}", tag=f"zT{g}")
                for g in range(NG)
            ]
            lhsT = singles.tile([P, RPC], bf16, tag="lhsT")
            zA = singles.tile([P, RPC], f32, tag="zA")
            zB = singles.tile([P, RPC], f32, tag="zB")
            ss_ab = singles.tile([P, 2 * MT], f32, tag="ss_ab")
            lns_ab = singles.tile([P, 2 * MT], f32, tag="lns_ab")
            inv_ab = singles.tile([P, 2 * MT], f32, tag="inv_ab")
            esums = singles.tile([P, MT * NG], f32, tag="esums")
            pos = singles.tile([P, MT], f32, tag="pos")
            svec = singles.tile([P, MT], f32, tag="svec")
            denoms = singles.tile([P, MT], f32, tag="denoms")
            lnb = singles.tile([P, MT], f32, tag="lnb")
            pos2 = singles.tile([P, MT], f32, tag="pos2")
            outb = singles.tile([P, MT], f32, tag="outb")

            # ---- load own + partner row blocks (natural layout) ----
            # row r = t*128 + p  ->  zA[p, t*128:(t+1)*128]
            nc.sync.dma_start(
                out=zA[:].rearrange("p (n d) -> p n d", d=D),
                in_=rows_a.ap().rearrange("(n p) d -> p n d", p=P),
            )
            nc.sync.dma_start(
                out=zB[:].rearrange("p (n d) -> p n d", d=D),
                in_=rows_b.ap().rearrange("(n p) d -> p n d", p=P),
            )

            # ---- normalize A/B in fp32 ----
            for t in range(2 * MT):
                src = zA if t < MT else zB
                sl = slice((t % MT) * D, (t % MT + 1) * D)
                scr = scratch.tile([P, D], f32, tag="scr")
                nc.vector.tensor_mul(scr[:], src[:, sl], src[:, sl])
                nc.vector.tensor_reduce(
                    ss_ab[:, t : t + 1], scr[:], axis=mybir.AxisListType.X, op=OP.add
                )
            nc.scalar.activation(lns_ab[:], ss_ab[:], AF.Ln)
            nc.scalar.activation(inv_ab[:], lns_ab[:], AF.Exp, scale=-0.5)
            for t in range(2 * MT):
                src = zA if t < MT else zB
                sl = slice((t % MT) * D, (t % MT + 1) * D)
                nc.vector.tensor_scalar_mul(src[:, sl], src[:, sl], inv_ab[:, t : t + 1])

            for m in range(MT):
                sl = slice(m * D, (m + 1) * D)
                # positives: pos[p, m] = sum_d zA[p, m, d] * zB[p, m, d]
                scr = scratch.tile([P, D], f32, tag="scr")
                nc.vector.tensor_mul(scr[:], zA[:, sl], zB[:, sl])
                nc.vector.tensor_reduce(
                    pos[:, m : m + 1], scr[:], axis=mybir.AxisListType.X, op=OP.add
                )
                # lhsT[:, m*128+j] = zA row j of tile m (transposed, cast bf16)
                pt = psum_t.tile([P, P], f32, tag="pt")
                nc.tensor.transpose(pt[:], zA[:, sl], ident[:])
                nc.vector.tensor_copy(lhsT[:, sl], pt[:])

            # ---- main pipeline over 8 groups of 1024 reps rows ----
            # Software-pipelined: group g+1's load/normalize/transpose is
            # emitted BEFORE group g's matmul+exp stage so the ACT queue
            # never stalls between exp batches (its small Ln/Exp norm ops
            # are queued ahead of the big exps that would otherwise block
            # the next group's whole dependency chain).
            reps_v = reps.ap().rearrange("(g n p) d -> g p n d", g=NG, p=P)

            def prep(g):
                ld = loads.tile([P, RPC], f32, tag="ld", name=f"ld{g}")
                nc.sync.dma_start(
                    out=ld[:].rearrange("p (n d) -> p n d", d=D), in_=reps_v[g]
                )
                ss = small.tile([P, MT], f32, tag="ss", name=f"ss{g}")
                for t in range(MT):
                    sl = slice(t * D, (t + 1) * D)
                    scr = scratch.tile([P, D], f32, tag="scr", name=f"scr{g}_{t}")
                    nc.vector.tensor_mul(scr[:], ld[:, sl], ld[:, sl])
                    nc.vector.tensor_reduce(
                        ss[:, t : t + 1], scr[:], axis=mybir.AxisListType.X, op=OP.add
                    )
                lns = small.tile([P, MT], f32, tag="lns", name=f"lns{g}")
                nc.scalar.activation(lns[:], ss[:], AF.Ln)
                inv = small.tile([P, MT], f32, tag="inv", name=f"inv{g}")
                nc.scalar.activation(inv[:], lns[:], AF.Exp, scale=-0.5)
                for t in range(MT):
                    sl = slice(t * D, (t + 1) * D)
                    nc.vector.tensor_scalar_mul(ld[:, sl], ld[:, sl], inv[:, t : t + 1])
                    pt = psum_t.tile([P, P], f32, tag="pt", name=f"pt{g}_{t}")
                    nc.tensor.transpose(pt[:], ld[:, sl], ident[:])
                    nc.vector.tensor_copy(zT[g][:, sl], pt[:])

            def mm_exp(g):
                for m in range(MT):
                    msl = slice(m * D, (m + 1) * D)
                    pg = psum_mm.tile([P, 1024], f32, tag="pg", name=f"pg{g}_{m}")
                    nc.tensor.matmul(
                        pg[:, 0:512], lhsT[:, msl], zT[g][:, 0:512],
                        start=True, stop=True,
                    )
                    nc.tensor.matmul(
                        pg[:, 512:1024], lhsT[:, msl], zT[g][:, 512:1024],
                        start=True, stop=True,
                    )
                    # exp(sim/T) in place on PSUM; row-sum into esums column
                    nc.scalar.activation(
                        pg[:], pg[:], AF.Exp, scale=INV_T,
                        accum_out=esums[:, m * NG + g : m * NG + g + 1],
                    )

            prep(0)
            for g in range(NG):
                if g + 1 < NG:
                    prep(g + 1)
                mm_exp(g)

            # ---- finale: loss terms per local row ----
            for m in range(MT):
                nc.vector.tensor_reduce(
                    svec[:, m : m + 1], esums[:, m * NG : (m + 1) * NG],
                    axis=mybir.AxisListType.X, op=OP.add,
                )
            nc.vector.tensor_scalar_add(denoms[:], svec[:], -E2)
            nc.scalar.activation(lnb[:], denoms[:], AF.Ln)
            nc.vector.tensor_scalar_mul(pos2[:], pos[:], INV_T)
            nc.vector.tensor_tensor(outb[:], lnb[:], pos2[:], OP.subtract)
            nc.sync.dma_start(out=out_d.ap(), in_=outb[:])

    _hoist_excess_waits(nc)
    return nc


def _get_nc() -> bass.Bass:
    global _NC
    if _NC is None:
        _NC = _build_nc()
    return _NC


def kernel(emb_i: np.ndarray, emb_j: np.ndarray) -> np.ndarray:
    global _LAST_RESULT
    reps = np.ascontiguousarray(
        np.concatenate(
            [np.asarray(emb_i, np.float32), np.asarray(emb_j, np.float32)], axis=0
        )
    )
    assert reps.shape == (NR, D)

    in_maps = []
    for c in range(N_CORES):
        lo = c * RPC
        pa = (lo + B) % NR
        in_maps.append(
            {
                "reps": reps,
                "rows_a": np.ascontiguousarray(reps[lo : lo + RPC]),
                "rows_b": np.ascontiguousarray(reps[pa : pa + RPC]),
            }
        )

    kw = {}
    if TRACE:
        import os
        import tempfile

        kw["tmpdir"] = tempfile.mkdtemp(prefix="trace_", dir=os.getcwd())
    res = run_bass_kernel_spmd(
        _get_nc(), in_maps, list(range(N_CORES)), trace=TRACE, **kw
    )
    _LAST_RESULT = res

    total = 0.0
    for r in res.results:
        total += float(np.asarray(r["out"], dtype=np.float64).sum())
    return np.asarray(np.float32(total / NR))
